# revision 4
# baseline (speedup 1.0000x reference)
"""Trainium2 Bass kernel for nn_BoardLoss (Tango board loss), v6.

Reference semantics (per sample, 6x6 board, batch 2,000,000):
  b = (x > 0.5); a row/col counts 1 if it contains any run of 3 equal
  consecutive cells; loss = mean over batch of (row_hits + col_hits)/6.

Algorithm (exact integer arithmetic in bf16):
  s = Sign(x - (0.5 + 2^-24)) in {-1,+1}
      (exact for every value jax.random.uniform emits: those sit on the
       2^-23 grid and the fp32 fma x*1 + bias is sign-exact)
  w = window sum of 3 consecutive cells of a line; |w| = 3 iff triple
  per line v = sum over its 4 windows of |w|  (v in {4..12})
  line has a triple  <=>  v >= 6
  device accumulates  Sign(v - 5) in {-1,+1}  over all lines;
  host: hits = (total + 12N)/2, loss = hits/(6N) = (total + 12N)/(12N).

Layout: pure data parallel over 8 cores (250k samples each). Within a
core, blocks of [128 partitions x G pairs x 2 samples]; each pair of
samples is cell-interleaved in SBUF (k in {0,1} innermost) so every
shifted window view keeps a [step-1, count>=2] 4B-aligned inner run
=> DVE 2x mode. All access patterns <= 3 free dims.

Engine split per block (G=96 pairs):
  ACT : one Sign op (transposing read interleaves the pair),
        and the final fused threshold+reduce: Sign(v-5) with accum_out
  DVE : 9 window-sum adds sharing the middle pairs
        (w0=c0+P1, w1=P1+c3, w2=c2+P3, w3=P3+c5; same vertically),
        |W| via one flat uint16 sign-bit AND (4x mode),
        2 adds for the per-line window sums v
"""

import numpy as np

import concourse.bacc as bacc
import concourse.mybir as mybir
from concourse.alu_op_type import AluOpType
from concourse.tile import TileContext
from concourse.bass_utils import run_bass_kernel_spmd

F32 = mybir.dt.float32
BF16 = mybir.dt.bfloat16
SIGN_BIAS = -(0.5 + 2.0**-24)
ADD = AluOpType.add

BATCH = 2_000_000
N_CORES = 8
N_PER_CORE = BATCH // N_CORES  # 250,000
G_MAX = 96
ABS_MODE = "split"  # "dve" (flat TS-and) or "split" (half on ACT)


def _plan_blocks(n_samples: int, g_max: int):
    """Blocks of (sample_base, p, G): p partitions x G pairs x 2 samples."""
    groups = n_samples // 2
    assert groups * 2 == n_samples
    rows = groups // 128
    tail = groups - rows * 128
    blocks = []
    base = 0
    r = rows
    while r > 0:
        g = min(g_max, r)
        blocks.append((base, 128, g))
        base += 128 * g * 2
        r -= g
    if tail:
        blocks.append((base, tail, 1))
    return blocks


def _build(n_per_core: int, g_max: int, repeat: int = 1):
    nc = bacc.Bacc()
    blocks = _plan_blocks(n_per_core, g_max)
    n_blocks = len(blocks)

    x_in = nc.dram_tensor("x", [n_per_core, 36], F32, kind="ExternalInput")
    out_d = nc.dram_tensor("partial", [128, n_blocks], F32, kind="ExternalOutput")

    with TileContext(nc) as tc:
        with (
            tc.tile_pool(name="xin", bufs=2) as xin_pool,
            tc.tile_pool(name="sgn", bufs=2) as sgn_pool,
            tc.tile_pool(name="mid", bufs=2) as mid_pool,
            tc.tile_pool(name="singles", bufs=1) as singles,
        ):
            acc = singles.tile([128, n_blocks], F32)
            nc.vector.memset(acc, 0.0)
            bias_t = singles.tile([128, 1], F32)
            nc.vector.memset(bias_t, SIGN_BIAS)
            mask_t = singles.tile([128, 1], mybir.dt.uint16)
            nc.vector.memset(mask_t, 0x7FFF)
            zb = singles.tile([128, 1], F32)
            nc.vector.memset(zb, 0.0)
            neg5 = singles.tile([128, 1], F32)
            nc.vector.memset(neg5, -5.0)

            for rep in range(repeat):
              for blk, (base, p, g) in enumerate(blocks):
                x_t = xin_pool.tile([128, g, 2, 36], F32, tag="x")
                xv = x_in[base : base + p * g * 2, :].rearrange(
                    "(p g k) c -> p g k c", p=p, g=g, k=2
                )
                nc.sync.dma_start(out=x_t[:p], in_=xv)

                # ACT: threshold to +-1, pair-interleaved output
                s_t = sgn_pool.tile([128, g, 72], BF16, tag="s")
                sv = s_t.rearrange("p g (c k) -> p g c k", k=2)
                nc.scalar.activation(
                    sv[:p], x_t[:p].rearrange("p g k c -> p g c k"),
                    mybir.ActivationFunctionType.Sign, bias=bias_t[:p],
                )

                SR = s_t.rearrange("p g (r x) -> p g r x", r=6)

                def cell(j):  # [p, g, 6, 2]: column j of each row, both k
                    return SR[:p, :, :, 2 * j : 2 * j + 2]

                def rows(r0):  # [p, g, 2, 12]: rows {r0, r0+2}, both k
                    return SR[:p, :, r0 : r0 + 3 : 2, :]

                # DVE: 48 window sums / pair, sharing middle pairs
                # W[h, m, sect, line*k]: window w_{2m+h}; sect 0 rows, 1 cols
                W = mid_pool.tile([128, g, 2, 2, 2, 12], BF16, tag="W")
                Pr = mid_pool.tile([128, g, 2, 6, 2], BF16, tag="Pr")
                Qc = mid_pool.tile([128, g, 2, 12], BF16, tag="Qc")

                nc.vector.tensor_tensor(Pr[:p, :, 0], cell(1), cell(2), op=ADD)
                nc.vector.tensor_tensor(Pr[:p, :, 1], cell(3), cell(4), op=ADD)
                Wr = W.rearrange("p g h m s (r k) -> p g h m s r k", k=2)
                nc.vector.tensor_tensor(
                    Wr[:p, :, 0, 0, 0], cell(0), Pr[:p, :, 0], op=ADD)
                nc.vector.tensor_tensor(
                    Wr[:p, :, 0, 1, 0], cell(2), Pr[:p, :, 1], op=ADD)
                nc.vector.tensor_tensor(
                    Wr[:p, :, 1, 0, 0], Pr[:p, :, 0], cell(3), op=ADD)
                nc.vector.tensor_tensor(
                    Wr[:p, :, 1, 1, 0], Pr[:p, :, 1], cell(5), op=ADD)
                nc.vector.tensor_tensor(Qc[:p], rows(1), rows(2), op=ADD)
                nc.vector.tensor_tensor(
                    W[:p, :, 0, :, 1], rows(0), Qc[:p], op=ADD)
                nc.vector.tensor_tensor(
                    W[:p, :, 1, :, 1], Qc[:p], rows(3), op=ADD)

                # |W| via sign-bit AND (flat views -> DVE 4x mode)
                Wa = mid_pool.tile([128, g, 2, 48], BF16, tag="Wa")
                if ABS_MODE == "split":
                    gh = max(1, g // 2)
                    nc.scalar.activation(
                        Wa[:p, :gh].rearrange("p g h x -> p g (h x)"),
                        W[:p, :gh].rearrange("p g h m s x -> p g (h m s x)"),
                        mybir.ActivationFunctionType.Abs, bias=zb[:p],
                    )
                    if g > gh:
                        nc.vector.tensor_scalar(
                            Wa[:p, gh:].rearrange("p g h x -> p (g h x)")
                                .bitcast(mybir.dt.uint16),
                            W[:p, gh:].rearrange(
                                "p g h m s x -> p (g h m s x)")
                                .bitcast(mybir.dt.uint16),
                            mask_t[:p], None, op0=AluOpType.bitwise_and,
                        )
                else:
                    nc.vector.tensor_scalar(
                        Wa[:p].rearrange("p g h x -> p (g h x)")
                            .bitcast(mybir.dt.uint16),
                        W[:p].rearrange("p g h m s x -> p (g h m s x)")
                            .bitcast(mybir.dt.uint16),
                        mask_t[:p], None, op0=AluOpType.bitwise_and,
                    )

                # v = sum over the 4 windows of each line
                S1 = mid_pool.tile([128, g, 48], BF16, tag="S1")
                nc.vector.tensor_tensor(
                    S1[:p], Wa[:p, :, 0], Wa[:p, :, 1], op=ADD)
                S2 = mid_pool.tile([128, g, 24], BF16, tag="S2")
                nc.vector.tensor_tensor(
                    S2[:p], S1[:p, :, 0:24], S1[:p, :, 24:48], op=ADD)

                # ACT: line indicator Sign(v-5), summed into acc[:, blk]
                thr = mid_pool.tile([128, g, 24], BF16, tag="thr")
                nc.scalar.activation(
                    thr[:p], S2[:p], mybir.ActivationFunctionType.Sign,
                    bias=neg5[:p], accum_out=acc[:p, blk : blk + 1],
                )

            nc.sync.dma_start(out=out_d[:, :], in_=acc)

    nc.finalize()
    return nc


_NC_CACHE = {}


def _get_nc():
    key = (N_PER_CORE, G_MAX)
    if key not in _NC_CACHE:
        _NC_CACHE[key] = _build(*key)
    return _NC_CACHE[key]


def _run(x: np.ndarray, **spmd_kwargs):
    """x: [2_000_000, 6, 6] float32. Returns (loss, BassKernelResults)."""
    assert x.shape == (BATCH, 6, 6) and x.dtype == np.float32
    x2 = np.ascontiguousarray(x).reshape(BATCH, 36)
    in_maps = [
        {"x": x2[c * N_PER_CORE : (c + 1) * N_PER_CORE]} for c in range(N_CORES)
    ]
    nc = _get_nc()
    res = run_bass_kernel_spmd(nc, in_maps, core_ids=list(range(N_CORES)),
                               **spmd_kwargs)
    total = float(sum(r["partial"].astype(np.float64).sum() for r in res.results))
    n_lines = 12.0 * BATCH
    loss = (total + n_lines) / n_lines
    return np.array([loss], dtype=np.float32), res


def kernel(x: np.ndarray) -> np.ndarray:
    x = np.asarray(x, dtype=np.float32)
    loss, _ = _run(x)
    return loss
